# revision 10
# baseline (speedup 1.0000x reference)
"""CenterLoss forward on 8 Trainium2 NeuronCores.

Reference semantics:
    distmat[b, c] = ||x_b||^2 + ||center_c||^2 - 2 <x_b, center_c>
    loss = sum(clip(distmat * onehot(labels), 1e-12, 1e12)) / B

The masked matrix is zero everywhere except (b, labels[b]), and clip() lifts
each of the B*(C-1) zeros to exactly 1e-12.  So:

    loss = ( sum_b clip(||x_b - centers[labels[b]]||^2, 1e-12, 1e12)
             + B*(C-1)*1e-12 ) / B

which needs only a row gather + per-row squared distance, not the full
(B, C) distance matrix.  (For this problem's inputs every ||x_b - c||^2 is
~1024 >> 1e-12, so the row clip provably never binds and partial sums can
be accumulated on-device.)

v5 device kernel (raw Bass, SPMD data-parallel over batch), latency-optimized:
  - HW-measured facts driving the design (see traces in /tmp/trace_*):
      * indirect_dma_start can only pair ONE offset per partition per
        instruction (the Q7 ucode transfers the whole per-partition out span
        per offset), so a 512-row gather needs 4 serialized instructions at
        ~1.4us of descriptor-gen each -> use InstDMAGatherAnt (dma_gather)
        instead, which is purpose-built for N-row gathers.
      * SWDGE drain rate ~90-100ns/descriptor spread over 16 SDMA engines;
        DVE runs 1 bf16 elem/cycle/lane (~0.52ns/col for [128, .] tiles).
      * a DMA's completion sem lands ~1.5-2us after its data when the ring
        sees no later descriptors; a trailing dummy DMA flushes receipts.
  - centers baked into the NEFF as a Const bf16 table [10000, 640]:
    cols 0..511 = centers, col 512 = ||center||^2 (csq), 513..639 = zero pad
    (row pitch 1280B and elem bytes 1152 both satisfy dma_gather's 256B
    alignment rules).
  - x fed pre-augmented on host as bf16 chunks of 513 columns:
      x_aug[p, t*513 + d] = -2 * x[t*128+p, d]   (d < 512)
      x_aug[p, t*513 + 512] = 1.0
    so one fused product with a gathered augmented center row sums to
    -2<x,c> + csq, and Square(0.5 * x_aug) row-sums to ||x||^2 + 1.0.
  - engines:
      sync (HWDGE):   int16 label load, [16, 32] wrapped layout = only 16
                      tiny descriptors (completes ~1us earlier than the
                      [128, 4] i32 layout); result store at the end
      scalar (HWDGE): x_aug load on its own ring; then ACT Square+accum
                      computes acc0 = sum(x^2) + 1.0 per partition
                      (scale=0.5 folds away the -2), off the critical path
      gpsimd:         load_library(mlp); TWO dma_gathers of 256 rows each
                      (chunks 0-1, then 2-3) on queue 0, then a dummy DMA
                      chaser to flush the second gather's receipts
      vector (DVE):   per gather half, ONE fused STT
                        acc[1+h] = sum(x_aug * c_aug)  over [128, 2, 513]
                      (strided views skipping the gathered rows' pad cols;
                      HW-verified same DVE speed as contiguous)
  - per-core output: [128, 3] f32; host computes
    sum(acc0 - 1.0 + acc1 + acc2), adds B*(C-1)*1e-12, divides by B.
  - dma_gather index semantics (bass_interp + dma_gather.cpp): gathered
    element i lands at dst[i%128, i//128, :] and reads idxs[i%16, i//16],
    so host packs seq.reshape(16,16).T per gather half.
"""

import hashlib
from contextlib import ExitStack

import ml_dtypes
import numpy as np

import concourse.bass as bass
from concourse import mybir
from concourse.bass_utils import run_bass_kernel_spmd
from concourse.library_config import mlp as mlp_library

B = 4096
D = 512
C = 10000
NCORES = 8
BL = B // NCORES          # 512 rows per core
P = 128                   # partitions
NT = BL // P              # 4 chunks per core
DA = D + 1                # augmented row: centers row + csq
CW = 640                  # padded table row pitch (1280B, 256B-aligned)
EL = 640                  # gathered elements per row (1280B, 256B-aligned)
NH = 2                    # gather halves
TPH = NT // NH            # chunks per half

F32 = mybir.dt.float32
BF16 = mybir.dt.bfloat16
I16 = mybir.dt.int16

_CACHE = {}


def legalize_waits(nc, max_waits=1):
    """The walrus build in this container accepts at most one embedded
    sem-wait per TPB instruction ("Too many sync wait commands" otherwise).
    Split any excess into standalone single-wait InstEventSemaphore no-ops
    immediately before the instruction on the same engine — engine program
    order then enforces the identical synchronization."""
    n_split = 0
    for f in nc.m.functions:
        for b in f.blocks:
            insts = list(b.instructions)
            out = []
            for inst in insts:
                si = inst.sync_info
                waits = list(si.on_wait) if (si is not None and si.on_wait) else []
                if len(waits) > max_waits:
                    keep = waits[-max_waits:]
                    spill = waits[:-max_waits]
                    for k, w in enumerate(spill):
                        out.append(
                            mybir.InstEventSemaphore(
                                name=f"{inst.name}-lw{k}",
                                engine=inst.engine,
                                sync_info=mybir.SyncInfo(on_wait=[w], on_update=[]),
                            )
                        )
                        n_split += 1
                    inst.sync_info = mybir.SyncInfo(
                        on_wait=keep, on_update=list(si.on_update or [])
                    )
                out.append(inst)
            b.instructions = out
    return n_split


def embed_wait(bi, sem, val):
    """Attach a sem-ge wait directly to an instruction's sync_info instead of
    emitting a standalone EVENT_SEMAPHORE before it.  A standalone wait stalls
    the engine's prefetch of the next (large) instruction; an embedded wait
    lets fetch/decode/setup overlap the waiting.  With several waits,
    legalize_waits keeps the LAST appended wait embedded and spills earlier
    ones to standalone EVENT_SEMAPHOREs before the instruction."""
    ins = bi.ins
    si = ins.sync_info
    waits = list(si.on_wait or []) if si is not None else []
    upds = list(si.on_update or []) if si is not None else []
    waits.append(
        mybir.SyncWait(
            sync_type="semaphore",
            id=sem.num,
            ant_name=sem.name,
            wait_mode="sem-ge-imm",
            wait_value=val,
            wait_reg=None,
        )
    )
    ins.sync_info = mybir.SyncInfo(on_wait=waits, on_update=upds)
    return bi


def hoist_before_preamble(nc, inst_names):
    """Move the named instructions to the front of the main block, before the
    Bass-ctor const-AP memsets and all-engine barrier.  Only legal for
    instructions whose engine-side dependencies are register-free DMAs that
    touch no const APs: the owning engine then issues them ahead of its
    barrier arrival, overlapping the DMA latency with the preamble."""
    blk = nc.m.functions[0].blocks[0]
    insts = list(blk.instructions)
    moved = [i for i in insts if i.name in inst_names]
    rest = [i for i in insts if i.name not in inst_names]
    # keep the dummycall first (walrus uses it for the dge table)
    assert rest and type(rest[0]).__name__ == "InstCall"
    blk.instructions = [rest[0]] + moved + rest[1:]
    return len(moved)


def build_nc(centers_np):
    nc = bass.Bass(num_swdge_queues=2)

    x = nc.dram_tensor("x", [P, NT * DA], BF16, kind="ExternalInput")
    # [16, 32] int16 index block tiled 8x down the partitions: each Q7 core
    # reads its own 16-partition copy during descriptor generation.
    labels = nc.dram_tensor("labels", [P, BL // 16], I16, kind="ExternalInput")
    out = nc.dram_tensor("out", [P, 3], F32, kind="ExternalOutput")

    cen = np.ascontiguousarray(centers_np, dtype=np.float32)
    csq = np.sum(cen * cen, axis=1, dtype=np.float32)
    cen_aug = np.zeros((C, CW), dtype=ml_dtypes.bfloat16)
    cen_aug[:, :D] = cen.astype(ml_dtypes.bfloat16)
    cen_aug[:, D] = csq.astype(ml_dtypes.bfloat16)
    centers = nc.inline_tensor(np.ascontiguousarray(cen_aug), name="centers")

    es = ExitStack()
    idx_sb = es.enter_context(nc.sbuf_tensor("idx_sb", [P, BL // 16], I16))
    x_sb = es.enter_context(nc.sbuf_tensor("x_sb", [P, NT * DA], BF16))
    c_sb = es.enter_context(nc.sbuf_tensor("c_sb", [P, NT * EL], BF16))
    junka = es.enter_context(nc.sbuf_tensor("junka", [P, NT * DA], BF16))
    junkc = es.enter_context(nc.sbuf_tensor("junkc", [P, NT * DA], BF16))
    acc = es.enter_context(nc.sbuf_tensor("acc", [P, 3], F32))
    scr_sb = es.enter_context(nc.sbuf_tensor("scr_sb", [16, BL // 16], I16))
    idx_sem = es.enter_context(nc.semaphore("idx_sem"))
    x_sem = es.enter_context(nc.semaphore("x_sem"))
    c_sems = [es.enter_context(nc.semaphore(f"c_sem{h}")) for h in range(NH)]
    a_sem = es.enter_context(nc.semaphore("a_sem"))
    v_sem = es.enter_context(nc.semaphore("v_sem"))
    o_sem = es.enter_context(nc.semaphore("o_sem"))
    dve_sem = es.enter_context(nc.semaphore("dve_sem"))
    f_sem = es.enter_context(nc.semaphore("f_sem"))

    # ---- HWDGE loads, both hoisted pre-barrier; separate rings ----
    lab_dma = nc.sync.dma_start(out=idx_sb[:, :], in_=labels[:, :])
    lab_dma.then_inc(idx_sem, 16)
    x_dma = nc.scalar.dma_start(out=x_sb[:, :], in_=x[:, :])
    x_dma.then_inc(x_sem, 16)

    # ---- gpsimd: two 256-row dma_gathers on queue 0 ----
    nc.gpsimd.load_library(mlp_library)
    for h in range(NH):
        cols = slice(h * TPH * EL, (h + 1) * TPH * EL)
        g = nc.gpsimd.dma_gather(
            out_ap=c_sb[:, cols].rearrange("p (t d) -> p t d", t=TPH),
            in_ap=centers[:, 0:EL],
            idxs_ap=idx_sb[:, h * 16:(h + 1) * 16],
            num_idxs=NH * P,
            num_idxs_reg=NH * P,
            elem_size=EL,
            elem_step=CW,
        ).then_inc(c_sems[h], 16)
        if h == 0:
            embed_wait(g, idx_sem, 16)  # indices resident before gen
    # trailing dummy SWDGE DMA on the same queue flushes the second
    # gather's completion receipts promptly.
    nc.gpsimd.dma_start(out=scr_sb[0:1, :], in_=labels[0:1, :]).then_inc(f_sem, 16)

    # ---- scalar/ACT (off critical path): acc0 = sum(x^2) + 1.0 ----
    embed_wait(nc.scalar.activation(
        out=junka[:, :],
        in_=x_sb[:, :],
        func=mybir.ActivationFunctionType.Square,
        scale=0.5,
        accum_out=acc[:, 0:1],
    ).then_inc(a_sem, 1), x_sem, 16)

    # ---- vector: one fused STT per gather half ----
    # acc[1+h] = sum(x_aug * c_aug) over [128, TPH, 513] strided views
    for h in range(NH):
        xv = x_sb[:, h * TPH * DA:(h + 1) * TPH * DA].rearrange(
            "p (t d) -> p t d", t=TPH)
        jv = junkc[:, h * TPH * DA:(h + 1) * TPH * DA].rearrange(
            "p (t d) -> p t d", t=TPH)
        cv = c_sb[:, h * TPH * EL:(h + 1) * TPH * EL].rearrange(
            "p (t d) -> p t d", t=TPH)[:, :, 0:DA]
        stt = nc.vector.scalar_tensor_tensor(
            out=jv,
            in0=xv,
            scalar=1.0,
            in1=cv,
            op0=mybir.AluOpType.mult,
            op1=mybir.AluOpType.mult,
            accum_out=acc[:, 1 + h:2 + h],
        )
        if h == 0:
            # needs x AND the gather; x lands first in practice but is not
            # ordered — spill the x wait standalone, keep c embedded.
            embed_wait(stt, x_sem, 16)
        embed_wait(stt, c_sems[h], 16)
        stt.then_inc(v_sem if h == NH - 1 else dve_sem, 1)

    # ---- result out on the idle Sync HWDGE ring.  v_sem >= 1 proves both
    # STTs' accumulator drains (in-order DVE); a_sem >= 1 proves acc0. ----
    odma = nc.sync.dma_start(out=out[:, :], in_=acc[:, :]).then_inc(o_sem, 16)
    embed_wait(odma, a_sem, 1)   # spilled standalone (early, non-critical)
    embed_wait(odma, v_sem, 1)   # kept embedded (critical)

    # NOTE: the ExitStack is intentionally NOT closed — closing would free
    # the semaphores and emit an expensive end-of-program drain + barrier.
    hoist_before_preamble(nc, {lab_dma.ins.name, x_dma.ins.name})
    legalize_waits(nc)
    # Raw Bass skips Bacc's codegen_inst_isa_subclasses pass, so the
    # load_library pseudo's .instr words stay empty and walrus fails with
    # "ISA wrong length".  Run it by hand (see library_overlay.py).
    mybir.codegen_inst_isa_subclasses(nc)
    return nc


def _get_nc(centers_np):
    arr = np.ascontiguousarray(centers_np, np.float32)
    key = hashlib.md5(arr.tobytes()).hexdigest()
    if _CACHE.get("key") != key:
        _CACHE["nc"] = build_nc(arr)
        _CACHE["key"] = key
    return _CACHE["nc"]


def make_in_maps(x, labels, centers=None):
    x = np.asarray(x, dtype=np.float32)
    lab = np.asarray(labels).astype(np.int16).reshape(NCORES, NT, P)  # [c, T, p]
    # dma_gather index packing: gathered element i of half h reads
    # idxs[i%16, h*16 + i//16] and lands at dst[i%128, i//128]; element i
    # covers (T = h*TPH + i//128, p = i%128).
    idxs = np.empty((NCORES, 16, BL // 16), dtype=np.int16)
    for h in range(NH):
        seq = lab[:, h * TPH:(h + 1) * TPH, :].reshape(NCORES, NH * P)  # [c, i]
        idxs[:, :, h * 16:(h + 1) * 16] = seq.reshape(
            NCORES, 16, 16).transpose(0, 2, 1)
    idxs = np.ascontiguousarray(np.tile(idxs, (1, P // 16, 1)))  # replicate 8x
    # x transposed per core then augmented: [p, t*DA+d] = -2*x[t*128+p, d],
    # [p, t*DA+512] = 1.0
    xs = x.reshape(NCORES, NT, P, D).transpose(0, 2, 1, 3)  # [core, p, t, d]
    xa = np.empty((NCORES, P, NT, DA), dtype=np.float32)
    xa[..., :D] = -2.0 * xs
    xa[..., D] = 1.0
    xa = np.ascontiguousarray(
        xa.astype(ml_dtypes.bfloat16).reshape(NCORES, P, NT * DA)
    )
    return [{"x": xa[i], "labels": idxs[i]} for i in range(NCORES)]


def finalize(results):
    total = 0.0
    for r in results:
        vals = np.asarray(r["out"], dtype=np.float64)
        # acc0 = sum_t ||x_t||^2 + 1.0 (the four Square(0.5) constants),
        # acc1/acc2 = sum_t (csq_t - 2<x_t, c_t>) per half.  The row clip
        # never binds (all distances ~1e3), so summing before the clip
        # floor is exact.
        total += float((vals[:, 0] - 1.0 + vals[:, 1] + vals[:, 2]).sum())
    loss = (total + B * (C - 1) * 1e-12) / B
    return np.array(loss, dtype=np.float32)


def kernel(x, labels, centers):
    nc = _get_nc(centers)
    in_maps = make_in_maps(x, labels)
    res = run_bass_kernel_spmd(nc, in_maps, core_ids=list(range(NCORES)))
    return finalize(res.results)


# revision 12
# speedup vs baseline: 1.2315x; 1.2315x over previous
"""CenterLoss forward on 8 Trainium2 NeuronCores.

Reference semantics:
    distmat[b, c] = ||x_b||^2 + ||center_c||^2 - 2 <x_b, center_c>
    loss = sum(clip(distmat * onehot(labels), 1e-12, 1e12)) / B

The masked matrix is zero everywhere except (b, labels[b]), and clip() lifts
each of the B*(C-1) zeros to exactly 1e-12.  So:

    loss = ( sum_b clip(||x_b - centers[labels[b]]||^2, 1e-12, 1e12)
             + B*(C-1)*1e-12 ) / B

which needs only a row gather + per-row squared distance, not the full
(B, C) distance matrix.  (For this problem's inputs every ||x_b - c||^2 is
~1024 >> 1e-12, so the row clip provably never binds and partial sums can
be accumulated on-device.)

v6 device kernel (raw Bass, SPMD data-parallel over batch), latency-optimized.

HW-measured facts driving the design (traces under /tmp/trace_*):
  * indirect_dma_start pairs ONE offset per partition per instruction (the
    Q7 ucode transfers the whole per-partition out span per offset), so a
    512-row gather needs 4 serialized 128-row instructions; descriptor gen
    occupies the Pool engine ~1.4-1.6us per instruction.
  * InstDMAGatherAnt (dma_gather) would do it in fewer instructions BUT its
    library reload (MODIFY_POOL_CONFIG -> load_external_libraries) stalls
    the first custom instruction ~9us, and its gen is ~10ns/desc — net
    loss (measured 27.7us total).  Plain indirect DMA needs no reload.
  * SWDGE drain ~90ns/descriptor over 16 SDMA engines (~0.72us per 128
    1026B rows); DVE and ACT both run 1 bf16 elem/cycle/lane.
  * A SWDGE DMA's completion receipts only flush when LATER descriptors on
    the same ring are processed -> keep all gathers + one dummy chaser on
    ONE queue so each gather's sem fires as the next one drains.
  * An HWDGE DMA's completion sem lands ~1.0-1.5us after its data (write
    receipts); splitting the label load into four [128,1] DMAs lets gather
    t start as soon as ITS column's sem fires (~0.7us earlier than one
    [128,4] load).

Layout / math:
  - centers baked into the NEFF as a Const bf16 table [10000, 513]:
    cols 0..511 = centers, col 512 = ||center||^2 (csq).
  - x fed pre-augmented on host as bf16 chunks of 513 columns:
      x_aug[p, t*513 + d] = -2 * x[t*128+p, d]   (d < 512)
      x_aug[p, t*513 + 512] = 1.0
    so one fused product with a gathered augmented center row sums to
    -2<x,c> + csq, and ACT Square(0.5 * x_aug) row-sums to ||x||^2 + 1.0.
  - engines:
      sync (HWDGE):   labels for chunks 0,2 then x_aug; result store at end
      scalar (HWDGE): labels for chunks 1,3; then ACT Square+accum
                      acc0 = sum(x^2) + 1.0 per partition (scale=0.5 folds
                      away the -2), off the critical path
      gpsimd:         4 indirect 128-row gathers on queue 0 (each embeds a
                      wait on its own label column's sem), then one dummy
                      DMA chaser
      vector (DVE):   per chunk t, ONE fused STT
                        acc[1+t] = sum(x_aug_t * c_t)  (f32 accumulate)
  - per-core output: [128, 5] f32; host computes
    sum(acc0 - 1.0 + acc1 + acc2 + acc3 + acc4), adds B*(C-1)*1e-12,
    divides by B.
  - sync rules (validated on hardware):
      * every DMA whose completion matters gets its own semaphore
      * an STT's/ACT's then_inc fires after its accum_out drain
      * embedded (not standalone) waits let instruction fetch overlap the
        wait; walrus accepts at most one embedded wait per instruction
        (legalize_waits spills the rest; last appended wait stays embedded)
"""

import hashlib
from contextlib import ExitStack

import ml_dtypes
import numpy as np

import concourse.bass as bass
from concourse import mybir
from concourse.bass_utils import run_bass_kernel_spmd

B = 4096
D = 512
C = 10000
NCORES = 8
BL = B // NCORES          # 512 rows per core
P = 128                   # partitions
NT = BL // P              # 4 chunks per core
DA = D + 1                # augmented row: centers row + csq

F32 = mybir.dt.float32
BF16 = mybir.dt.bfloat16
I32 = mybir.dt.int32

_CACHE = {}


def legalize_waits(nc, max_waits=1):
    """The walrus build in this container accepts at most one embedded
    sem-wait per TPB instruction ("Too many sync wait commands" otherwise).
    Split any excess into standalone single-wait InstEventSemaphore no-ops
    immediately before the instruction on the same engine — engine program
    order then enforces the identical synchronization."""
    n_split = 0
    for f in nc.m.functions:
        for b in f.blocks:
            insts = list(b.instructions)
            out = []
            for inst in insts:
                si = inst.sync_info
                waits = list(si.on_wait) if (si is not None and si.on_wait) else []
                if len(waits) > max_waits:
                    keep = waits[-max_waits:]
                    spill = waits[:-max_waits]
                    for k, w in enumerate(spill):
                        out.append(
                            mybir.InstEventSemaphore(
                                name=f"{inst.name}-lw{k}",
                                engine=inst.engine,
                                sync_info=mybir.SyncInfo(on_wait=[w], on_update=[]),
                            )
                        )
                        n_split += 1
                    inst.sync_info = mybir.SyncInfo(
                        on_wait=keep, on_update=list(si.on_update or [])
                    )
                out.append(inst)
            b.instructions = out
    return n_split


def embed_wait(bi, sem, val):
    """Attach a sem-ge wait directly to an instruction's sync_info instead of
    emitting a standalone EVENT_SEMAPHORE before it.  A standalone wait stalls
    the engine's prefetch of the next (large) instruction; an embedded wait
    lets fetch/decode/setup overlap the waiting."""
    ins = bi.ins
    si = ins.sync_info
    waits = list(si.on_wait or []) if si is not None else []
    upds = list(si.on_update or []) if si is not None else []
    waits.append(
        mybir.SyncWait(
            sync_type="semaphore",
            id=sem.num,
            ant_name=sem.name,
            wait_mode="sem-ge-imm",
            wait_value=val,
            wait_reg=None,
        )
    )
    ins.sync_info = mybir.SyncInfo(on_wait=waits, on_update=upds)
    return bi


def hoist_before_preamble(nc, inst_names):
    """Move the named instructions to the front of the main block, before the
    Bass-ctor const-AP memsets and all-engine barrier.  Only legal for
    instructions whose engine-side dependencies are register-free DMAs that
    touch no const APs: the owning engine then issues them ahead of its
    barrier arrival, overlapping the DMA latency with the preamble."""
    blk = nc.m.functions[0].blocks[0]
    insts = list(blk.instructions)
    moved = [i for i in insts if i.name in inst_names]
    rest = [i for i in insts if i.name not in inst_names]
    # keep the dummycall first (walrus uses it for the dge table)
    assert rest and type(rest[0]).__name__ == "InstCall"
    blk.instructions = [rest[0]] + moved + rest[1:]
    return len(moved)


def build_nc(centers_np):
    nc = bass.Bass(num_swdge_queues=2)

    x = nc.dram_tensor("x", [P, NT * DA], BF16, kind="ExternalInput")
    # one [128, 1] label tensor per chunk: labN[p] = label[t*128 + p]
    labs = [nc.dram_tensor(f"labels{t}", [P, 1], I32, kind="ExternalInput")
            for t in range(NT)]
    out = nc.dram_tensor("out", [P, 1 + NT], F32, kind="ExternalOutput")

    cen = np.ascontiguousarray(centers_np, dtype=np.float32)
    csq = np.sum(cen * cen, axis=1, dtype=np.float32)
    cen_aug = np.concatenate([cen, csq[:, None]], axis=1).astype(ml_dtypes.bfloat16)
    centers = nc.inline_tensor(np.ascontiguousarray(cen_aug), name="centers")

    es = ExitStack()
    idx_sb = es.enter_context(nc.sbuf_tensor("idx_sb", [P, NT], I32))
    x_sb = es.enter_context(nc.sbuf_tensor("x_sb", [P, NT * DA], BF16))
    c_sb = es.enter_context(nc.sbuf_tensor("c_sb", [P, NT * DA], BF16))
    junka = es.enter_context(nc.sbuf_tensor("junka", [P, NT * DA], BF16))
    junkc = es.enter_context(nc.sbuf_tensor("junkc", [P, NT * DA], BF16))
    acc = es.enter_context(nc.sbuf_tensor("acc", [P, 1 + NT], F32))
    scr_sb = es.enter_context(nc.sbuf_tensor("scr_sb", [P, 1], I32))
    lab_sems = [es.enter_context(nc.semaphore(f"lab_sem{t}")) for t in range(NT)]
    x_sem = es.enter_context(nc.semaphore("x_sem"))
    c_sems = [es.enter_context(nc.semaphore(f"c_sem{t}")) for t in range(NT)]
    a_sem = es.enter_context(nc.semaphore("a_sem"))
    v_sem = es.enter_context(nc.semaphore("v_sem"))
    o_sem = es.enter_context(nc.semaphore("o_sem"))
    dve_sem = es.enter_context(nc.semaphore("dve_sem"))
    f_sem = es.enter_context(nc.semaphore("f_sem"))

    # ---- HWDGE loads, all hoisted pre-barrier.  Label columns alternate
    # across the two rings so the first two land concurrently; x follows on
    # the Sync ring. ----
    lab_dmas = []
    ring = [nc.sync, nc.scalar]
    for t in range(NT):
        ld = ring[t % 2].dma_start(out=idx_sb[:, t:t + 1], in_=labs[t][:, :])
        ld.then_inc(lab_sems[t], 16)
        lab_dmas.append(ld)
    x_dma = nc.sync.dma_start(out=x_sb[:, :], in_=x[:, :])
    x_dma.then_inc(x_sem, 16)

    # ---- gpsimd: 4 indirect 128-row gathers, all on queue 0 so each
    # gather's completion receipts are flushed by the next one's
    # descriptors; one dummy chaser flushes the last. ----
    for t in range(NT):
        gi = nc.gpsimd.indirect_dma_start(
            out=c_sb[:, t * DA:(t + 1) * DA],
            out_offset=None,
            in_=centers[:],
            in_offset=bass.IndirectOffsetOnAxis(ap=idx_sb[:, t:t + 1], axis=0),
        ).then_inc(c_sems[t], 16)
        embed_wait(gi, lab_sems[t], 16)
    nc.gpsimd.dma_start(out=scr_sb[0:1, :], in_=labs[0][0:1, :]).then_inc(f_sem, 16)

    # ---- scalar/ACT (off critical path): acc0 = sum(x^2) + 1.0 ----
    embed_wait(nc.scalar.activation(
        out=junka[:, :],
        in_=x_sb[:, :],
        func=mybir.ActivationFunctionType.Square,
        scale=0.5,
        accum_out=acc[:, 0:1],
    ).then_inc(a_sem, 1), x_sem, 16)

    # ---- vector: one fused accumulate-STT per chunk ----
    for t in range(NT):
        stt = nc.vector.scalar_tensor_tensor(
            out=junkc[:, t * DA:(t + 1) * DA],
            in0=x_sb[:, t * DA:(t + 1) * DA],
            scalar=1.0,
            in1=c_sb[:, t * DA:(t + 1) * DA],
            op0=mybir.AluOpType.mult,
            op1=mybir.AluOpType.mult,
            accum_out=acc[:, 1 + t:2 + t],
        )
        if t == 0:
            # needs x AND the gather; the x wait spills standalone.
            embed_wait(stt, x_sem, 16)
        embed_wait(stt, c_sems[t], 16)
        stt.then_inc(v_sem if t == NT - 1 else dve_sem, 1)

    # ---- result out on the Sync HWDGE ring (idle by then).  v_sem >= 1
    # proves all four STT accum drains (in-order DVE); a_sem >= 1 proves
    # acc0.  The a_sem wait spills standalone (early, non-critical). ----
    odma = nc.sync.dma_start(out=out[:, :], in_=acc[:, :]).then_inc(o_sem, 16)
    embed_wait(odma, a_sem, 1)
    embed_wait(odma, v_sem, 1)

    # NOTE: the ExitStack is intentionally NOT closed — closing would free
    # the semaphores and emit an expensive end-of-program drain + barrier.
    hoist_before_preamble(
        nc, {ld.ins.name for ld in lab_dmas} | {x_dma.ins.name})
    legalize_waits(nc)
    return nc


def _get_nc(centers_np):
    arr = np.ascontiguousarray(centers_np, np.float32)
    key = hashlib.md5(arr.tobytes()).hexdigest()
    if _CACHE.get("key") != key:
        _CACHE["nc"] = build_nc(arr)
        _CACHE["key"] = key
    return _CACHE["nc"]


def make_in_maps(x, labels, centers=None):
    x = np.asarray(x, dtype=np.float32)
    # [p, t] = label[t*128 + p] within each core's 512-row shard
    labels_i32 = np.ascontiguousarray(
        np.asarray(labels).astype(np.int32).reshape(NCORES, NT, P)
    )  # [core, t, p]
    # x transposed per core then augmented: [p, t*DA+d] = -2*x[t*128+p, d],
    # [p, t*DA+512] = 1.0
    xs = x.reshape(NCORES, NT, P, D).transpose(0, 2, 1, 3)  # [core, p, t, d]
    xa = np.empty((NCORES, P, NT, DA), dtype=np.float32)
    xa[..., :D] = -2.0 * xs
    xa[..., D] = 1.0
    xa = np.ascontiguousarray(
        xa.astype(ml_dtypes.bfloat16).reshape(NCORES, P, NT * DA)
    )
    maps = []
    for i in range(NCORES):
        m = {"x": xa[i]}
        for t in range(NT):
            m[f"labels{t}"] = np.ascontiguousarray(labels_i32[i, t][:, None])
        maps.append(m)
    return maps


def finalize(results):
    total = 0.0
    for r in results:
        vals = np.asarray(r["out"], dtype=np.float64)
        # acc0 = sum_t ||x_t||^2 + 1.0 (the four Square(0.5) constants),
        # acc[1..4] = csq_t - 2<x_t, c_t> per chunk.  The row clip never
        # binds (all distances ~1e3), so summing before the clip floor is
        # exact.
        total += float((vals[:, 0] - 1.0 + vals[:, 1:].sum(axis=1)).sum())
    loss = (total + B * (C - 1) * 1e-12) / B
    return np.array(loss, dtype=np.float32)


def kernel(x, labels, centers):
    nc = _get_nc(centers)
    in_maps = make_in_maps(x, labels)
    res = run_bass_kernel_spmd(nc, in_maps, core_ids=list(range(NCORES)))
    return finalize(res.results)


# revision 13
# speedup vs baseline: 1.5320x; 1.2441x over previous
"""CenterLoss forward on 8 Trainium2 NeuronCores.

Reference semantics:
    distmat[b, c] = ||x_b||^2 + ||center_c||^2 - 2 <x_b, center_c>
    loss = sum(clip(distmat * onehot(labels), 1e-12, 1e12)) / B

The masked matrix is zero everywhere except (b, labels[b]), and clip() lifts
each of the B*(C-1) zeros to exactly 1e-12.  So:

    loss = ( sum_b clip(||x_b - centers[labels[b]]||^2, 1e-12, 1e12)
             + B*(C-1)*1e-12 ) / B

which needs only a row gather + per-row squared distance, not the full
(B, C) distance matrix.  (For this problem's inputs every ||x_b - c||^2 is
~1024 >> 1e-12, so the row clip provably never binds and partial sums can
be accumulated on-device.)

v6 device kernel (raw Bass, SPMD data-parallel over batch), latency-optimized.

HW-measured facts driving the design (traces under /tmp/trace_*):
  * indirect_dma_start pairs ONE offset per partition per instruction (the
    Q7 ucode transfers the whole per-partition out span per offset), so a
    512-row gather needs 4 serialized 128-row instructions; descriptor gen
    occupies the Pool engine ~1.4-1.6us per instruction.
  * InstDMAGatherAnt (dma_gather) would do it in fewer instructions BUT its
    library reload (MODIFY_POOL_CONFIG -> load_external_libraries) stalls
    the first custom instruction ~9us, and its gen is ~10ns/desc — net
    loss (measured 27.7us total).  Plain indirect DMA needs no reload.
  * SWDGE drain ~90ns/descriptor over 16 SDMA engines (~0.72us per 128
    1026B rows); DVE and ACT both run 1 bf16 elem/cycle/lane.
  * A SWDGE DMA's completion receipts only flush when LATER descriptors on
    the same ring are processed -> keep all gathers + one dummy chaser on
    ONE queue so each gather's sem fires as the next one drains.
  * An HWDGE DMA's completion sem lands ~1.0-1.5us after its data (write
    receipts); splitting the label load into four [128,1] DMAs lets gather
    t start as soon as ITS column's sem fires (~0.7us earlier than one
    [128,4] load).

Layout / math:
  - centers baked into the NEFF as a Const bf16 table [10000, 513]:
    cols 0..511 = centers, col 512 = ||center||^2 (csq).
  - x fed pre-augmented on host as bf16 chunks of 513 columns:
      x_aug[p, t*513 + d] = -2 * x[t*128+p, d]   (d < 512)
      x_aug[p, t*513 + 512] = 1.0
    so one fused product with a gathered augmented center row sums to
    -2<x,c> + csq, and ACT Square(0.5 * x_aug) row-sums to ||x||^2 + 1.0.
  - engines:
      sync (HWDGE):   labels for chunks 0,2 then x_aug; result store at end
      scalar (HWDGE): labels for chunks 1,3; then ACT Square+accum
                      acc0 = sum(x^2) + 1.0 per partition (scale=0.5 folds
                      away the -2), off the critical path
      gpsimd:         4 indirect 128-row gathers on queue 0 (each embeds a
                      wait on its own label column's sem), then one dummy
                      DMA chaser
      vector (DVE):   per chunk t, ONE fused STT
                        acc[1+t] = sum(x_aug_t * c_t)  (f32 accumulate)
  - per-core output: [128, 5] f32; host computes
    sum(acc0 - 1.0 + acc1 + acc2 + acc3 + acc4), adds B*(C-1)*1e-12,
    divides by B.
  - sync rules (validated on hardware):
      * every DMA whose completion matters gets its own semaphore
      * an STT's/ACT's then_inc fires after its accum_out drain
      * embedded (not standalone) waits let instruction fetch overlap the
        wait; walrus accepts at most one embedded wait per instruction
        (legalize_waits spills the rest; last appended wait stays embedded)
"""

import hashlib
from contextlib import ExitStack

import ml_dtypes
import numpy as np

import concourse.bass as bass
from concourse import mybir
from concourse.bass_utils import run_bass_kernel_spmd

B = 4096
D = 512
C = 10000
NCORES = 8
BL = B // NCORES          # 512 rows per core
P = 128                   # partitions
NT = BL // P              # 4 chunks per core
DA = D + 1                # augmented row: centers row + csq

F32 = mybir.dt.float32
BF16 = mybir.dt.bfloat16
I32 = mybir.dt.int32

_CACHE = {}


def legalize_waits(nc, max_waits=1):
    """The walrus build in this container accepts at most one embedded
    sem-wait per TPB instruction ("Too many sync wait commands" otherwise).
    Split any excess into standalone single-wait InstEventSemaphore no-ops
    immediately before the instruction on the same engine — engine program
    order then enforces the identical synchronization."""
    n_split = 0
    for f in nc.m.functions:
        for b in f.blocks:
            insts = list(b.instructions)
            out = []
            for inst in insts:
                si = inst.sync_info
                waits = list(si.on_wait) if (si is not None and si.on_wait) else []
                if len(waits) > max_waits:
                    keep = waits[-max_waits:]
                    spill = waits[:-max_waits]
                    for k, w in enumerate(spill):
                        out.append(
                            mybir.InstEventSemaphore(
                                name=f"{inst.name}-lw{k}",
                                engine=inst.engine,
                                sync_info=mybir.SyncInfo(on_wait=[w], on_update=[]),
                            )
                        )
                        n_split += 1
                    inst.sync_info = mybir.SyncInfo(
                        on_wait=keep, on_update=list(si.on_update or [])
                    )
                out.append(inst)
            b.instructions = out
    return n_split


def embed_wait(bi, sem, val):
    """Attach a sem-ge wait directly to an instruction's sync_info instead of
    emitting a standalone EVENT_SEMAPHORE before it.  A standalone wait stalls
    the engine's prefetch of the next (large) instruction; an embedded wait
    lets fetch/decode/setup overlap the waiting."""
    ins = bi.ins
    si = ins.sync_info
    waits = list(si.on_wait or []) if si is not None else []
    upds = list(si.on_update or []) if si is not None else []
    waits.append(
        mybir.SyncWait(
            sync_type="semaphore",
            id=sem.num,
            ant_name=sem.name,
            wait_mode="sem-ge-imm",
            wait_value=val,
            wait_reg=None,
        )
    )
    ins.sync_info = mybir.SyncInfo(on_wait=waits, on_update=upds)
    return bi


def hoist_before_preamble(nc, inst_names):
    """Move the named instructions to the front of the main block, before the
    Bass-ctor const-AP memsets and all-engine barrier.  Only legal for
    instructions whose engine-side dependencies are register-free DMAs that
    touch no const APs: the owning engine then issues them ahead of its
    barrier arrival, overlapping the DMA latency with the preamble."""
    blk = nc.m.functions[0].blocks[0]
    insts = list(blk.instructions)
    moved = [i for i in insts if i.name in inst_names]
    rest = [i for i in insts if i.name not in inst_names]
    # keep the dummycall first (walrus uses it for the dge table)
    assert rest and type(rest[0]).__name__ == "InstCall"
    blk.instructions = [rest[0]] + moved + rest[1:]
    return len(moved)


def build_nc(centers_np):
    nc = bass.Bass(num_swdge_queues=2)

    x = nc.dram_tensor("x", [P, NT * DA], BF16, kind="ExternalInput")
    # one [128, 1] label tensor per chunk: labN[p] = label[t*128 + p]
    labs = [nc.dram_tensor(f"labels{t}", [P, 1], I32, kind="ExternalInput")
            for t in range(NT)]
    out = nc.dram_tensor("out", [P, 1 + NT], F32, kind="ExternalOutput")

    cen = np.ascontiguousarray(centers_np, dtype=np.float32)
    csq = np.sum(cen * cen, axis=1, dtype=np.float32)
    cen_aug = np.concatenate([cen, csq[:, None]], axis=1).astype(ml_dtypes.bfloat16)
    centers = nc.inline_tensor(np.ascontiguousarray(cen_aug), name="centers")

    es = ExitStack()
    idx_sb = es.enter_context(nc.sbuf_tensor("idx_sb", [P, NT], I32))
    x_sb = es.enter_context(nc.sbuf_tensor("x_sb", [P, NT * DA], BF16))
    c_sb = es.enter_context(nc.sbuf_tensor("c_sb", [P, NT * DA], BF16))
    junka = es.enter_context(nc.sbuf_tensor("junka", [P, NT * DA], BF16))
    junkc = es.enter_context(nc.sbuf_tensor("junkc", [P, NT * DA], BF16))
    acc = es.enter_context(nc.sbuf_tensor("acc", [P, 1 + NT], F32))
    scr_sb = es.enter_context(nc.sbuf_tensor("scr_sb", [P, 1], I32))
    lab_sems = [es.enter_context(nc.semaphore(f"lab_sem{t}")) for t in range(NT)]
    x_sem = es.enter_context(nc.semaphore("x_sem"))
    c_sems = [es.enter_context(nc.semaphore(f"c_sem{t}")) for t in range(NT)]
    a_sem = es.enter_context(nc.semaphore("a_sem"))
    v_sem = es.enter_context(nc.semaphore("v_sem"))
    o_sem = es.enter_context(nc.semaphore("o_sem"))
    dve_sem = es.enter_context(nc.semaphore("dve_sem"))
    f_sem = es.enter_context(nc.semaphore("f_sem"))

    # ---- HWDGE loads, all hoisted pre-barrier.  Label columns alternate
    # across the two rings so the first two land concurrently; x follows on
    # the Sync ring. ----
    lab_dmas = []
    ring = [nc.sync, nc.scalar]
    for t in range(NT):
        ld = ring[t % 2].dma_start(out=idx_sb[:, t:t + 1], in_=labs[t][:, :])
        ld.then_inc(lab_sems[t], 16)
        lab_dmas.append(ld)
    x_dma = nc.sync.dma_start(out=x_sb[:, :], in_=x[:, :])
    x_dma.then_inc(x_sem, 16)

    # ---- gpsimd: 4 indirect 128-row gathers, all on queue 0 so each
    # gather's completion receipts are flushed by the next one's
    # descriptors; one dummy chaser flushes the last. ----
    for t in range(NT):
        gi = nc.gpsimd.indirect_dma_start(
            out=c_sb[:, t * DA:(t + 1) * DA],
            out_offset=None,
            in_=centers[:],
            in_offset=bass.IndirectOffsetOnAxis(ap=idx_sb[:, t:t + 1], axis=0),
        ).then_inc(c_sems[t], 16)
        embed_wait(gi, lab_sems[t], 16)
    nc.gpsimd.dma_start(out=scr_sb[0:1, :], in_=labs[0][0:1, :]).then_inc(f_sem, 16)

    # ---- scalar/ACT (off critical path): acc0 = sum(x^2) + 1.0 ----
    embed_wait(nc.scalar.activation(
        out=junka[:, :],
        in_=x_sb[:, :],
        func=mybir.ActivationFunctionType.Square,
        scale=0.5,
        accum_out=acc[:, 0:1],
    ).then_inc(a_sem, 1), x_sem, 16)

    # ---- vector: one fused accumulate-STT per chunk ----
    for t in range(NT):
        stt = nc.vector.scalar_tensor_tensor(
            out=junkc[:, t * DA:(t + 1) * DA],
            in0=x_sb[:, t * DA:(t + 1) * DA],
            scalar=1.0,
            in1=c_sb[:, t * DA:(t + 1) * DA],
            op0=mybir.AluOpType.mult,
            op1=mybir.AluOpType.mult,
            accum_out=acc[:, 1 + t:2 + t],
        )
        if t == 0:
            # needs x AND the gather; the x wait spills standalone.
            embed_wait(stt, x_sem, 16)
        embed_wait(stt, c_sems[t], 16)
        stt.then_inc(v_sem if t == NT - 1 else dve_sem, 1)

    # ---- result out on the Sync HWDGE ring (idle by then).  v_sem >= 1
    # proves all four STT accum drains (in-order DVE); a_sem >= 1 proves
    # acc0.  The a_sem wait spills standalone (early, non-critical). ----
    odma = nc.sync.dma_start(out=out[:, :], in_=acc[:, :]).then_inc(o_sem, 16)
    embed_wait(odma, a_sem, 1)
    embed_wait(odma, v_sem, 1)

    # NOTE: the ExitStack is intentionally NOT closed — closing would free
    # the semaphores and emit an expensive end-of-program drain + barrier.
    hoist_before_preamble(
        nc, {ld.ins.name for ld in lab_dmas} | {x_dma.ins.name})
    # Drop the ctor's all-engine barrier: the hoisted label/x DMAs delay
    # every engine's barrier arrival to ~8.8us, which in turn delays Pool's
    # first gather fetch past the label sems.  All cross-engine ordering in
    # this program is via data semaphores; the only barrier-protected state
    # is the const-AP memsets (used solely by ACT's bias const, first read
    # at x_sem ~11us >> memset completion ~6.7us).  The follower DRAINs
    # keep their embedded arrival updates (harmless; sems never reused).
    for f in nc.m.functions:
        for blk in f.blocks:
            blk.instructions = [
                i for i in blk.instructions if "barrier_" not in i.name
            ]
    legalize_waits(nc)
    return nc


def _get_nc(centers_np):
    arr = np.ascontiguousarray(centers_np, np.float32)
    key = hashlib.md5(arr.tobytes()).hexdigest()
    if _CACHE.get("key") != key:
        _CACHE["nc"] = build_nc(arr)
        _CACHE["key"] = key
    return _CACHE["nc"]


def make_in_maps(x, labels, centers=None):
    x = np.asarray(x, dtype=np.float32)
    # [p, t] = label[t*128 + p] within each core's 512-row shard
    labels_i32 = np.ascontiguousarray(
        np.asarray(labels).astype(np.int32).reshape(NCORES, NT, P)
    )  # [core, t, p]
    # x transposed per core then augmented: [p, t*DA+d] = -2*x[t*128+p, d],
    # [p, t*DA+512] = 1.0
    xs = x.reshape(NCORES, NT, P, D).transpose(0, 2, 1, 3)  # [core, p, t, d]
    xa = np.empty((NCORES, P, NT, DA), dtype=np.float32)
    xa[..., :D] = -2.0 * xs
    xa[..., D] = 1.0
    xa = np.ascontiguousarray(
        xa.astype(ml_dtypes.bfloat16).reshape(NCORES, P, NT * DA)
    )
    maps = []
    for i in range(NCORES):
        m = {"x": xa[i]}
        for t in range(NT):
            m[f"labels{t}"] = np.ascontiguousarray(labels_i32[i, t][:, None])
        maps.append(m)
    return maps


def finalize(results):
    total = 0.0
    for r in results:
        vals = np.asarray(r["out"], dtype=np.float64)
        # acc0 = sum_t ||x_t||^2 + 1.0 (the four Square(0.5) constants),
        # acc[1..4] = csq_t - 2<x_t, c_t> per chunk.  The row clip never
        # binds (all distances ~1e3), so summing before the clip floor is
        # exact.
        total += float((vals[:, 0] - 1.0 + vals[:, 1:].sum(axis=1)).sum())
    loss = (total + B * (C - 1) * 1e-12) / B
    return np.array(loss, dtype=np.float32)


def kernel(x, labels, centers):
    nc = _get_nc(centers)
    in_maps = make_in_maps(x, labels)
    res = run_bass_kernel_spmd(nc, in_maps, core_ids=list(range(NCORES)))
    return finalize(res.results)


# revision 15
# speedup vs baseline: 1.5369x; 1.0032x over previous
"""CenterLoss forward on 8 Trainium2 NeuronCores.

Reference semantics:
    distmat[b, c] = ||x_b||^2 + ||center_c||^2 - 2 <x_b, center_c>
    loss = sum(clip(distmat * onehot(labels), 1e-12, 1e12)) / B

The masked matrix is zero everywhere except (b, labels[b]), and clip() lifts
each of the B*(C-1) zeros to exactly 1e-12.  So:

    loss = ( sum_b clip(||x_b - centers[labels[b]]||^2, 1e-12, 1e12)
             + B*(C-1)*1e-12 ) / B

which needs only a row gather + per-row squared distance, not the full
(B, C) distance matrix.  (For this problem's inputs every ||x_b - c||^2 is
~1024 >> 1e-12, so the row clip provably never binds and partial sums can
be accumulated on-device.)

v6 device kernel (raw Bass, SPMD data-parallel over batch), latency-optimized.

HW-measured facts driving the design (traces under /tmp/trace_*):
  * indirect_dma_start pairs ONE offset per partition per instruction (the
    Q7 ucode transfers the whole per-partition out span per offset), so a
    512-row gather needs 4 serialized 128-row instructions; descriptor gen
    occupies the Pool engine ~1.4-1.6us per instruction.
  * InstDMAGatherAnt (dma_gather) would do it in fewer instructions BUT its
    library reload (MODIFY_POOL_CONFIG -> load_external_libraries) stalls
    the first custom instruction ~9us, and its gen is ~10ns/desc — net
    loss (measured 27.7us total).  Plain indirect DMA needs no reload.
  * SWDGE drain ~90ns/descriptor over 16 SDMA engines (~0.72us per 128
    1026B rows); DVE and ACT both run 1 bf16 elem/cycle/lane.
  * A SWDGE DMA's completion receipts only flush when LATER descriptors on
    the same ring are processed -> keep all gathers + one dummy chaser on
    ONE queue so each gather's sem fires as the next one drains.
  * An HWDGE DMA's completion sem lands ~1.0-1.5us after its data (write
    receipts); splitting the label load into four [128,1] DMAs lets gather
    t start as soon as ITS column's sem fires (~0.7us earlier than one
    [128,4] load).

Layout / math:
  - centers baked into the NEFF as a Const bf16 table [10000, 513]:
    cols 0..511 = centers, col 512 = ||center||^2 (csq).
  - x fed pre-augmented on host as bf16 chunks of 513 columns:
      x_aug[p, t*513 + d] = -2 * x[t*128+p, d]   (d < 512)
      x_aug[p, t*513 + 512] = 1.0
    so one fused product with a gathered augmented center row sums to
    -2<x,c> + csq, and ACT Square(0.5 * x_aug) row-sums to ||x||^2 + 1.0.
  - engines:
      sync (HWDGE):   labels for chunks 0,2 then x_aug; result store at end
      scalar (HWDGE): labels for chunks 1,3; then ACT Square+accum
                      acc0 = sum(x^2) + 1.0 per partition (scale=0.5 folds
                      away the -2), off the critical path
      gpsimd:         4 indirect 128-row gathers on queue 0 (each embeds a
                      wait on its own label column's sem), then one dummy
                      DMA chaser
      vector (DVE):   per chunk t, ONE fused STT
                        acc[1+t] = sum(x_aug_t * c_t)  (f32 accumulate)
  - per-core output: [128, 5] f32; host computes
    sum(acc0 - 1.0 + acc1 + acc2 + acc3 + acc4), adds B*(C-1)*1e-12,
    divides by B.
  - sync rules (validated on hardware):
      * every DMA whose completion matters gets its own semaphore
      * an STT's/ACT's then_inc fires after its accum_out drain
      * embedded (not standalone) waits let instruction fetch overlap the
        wait; walrus accepts at most one embedded wait per instruction
        (legalize_waits spills the rest; last appended wait stays embedded)
"""

import hashlib
from contextlib import ExitStack

import ml_dtypes
import numpy as np

import concourse.bass as bass
from concourse import mybir
from concourse.bass_utils import run_bass_kernel_spmd

B = 4096
D = 512
C = 10000
NCORES = 8
BL = B // NCORES          # 512 rows per core
P = 128                   # partitions
NT = BL // P              # 4 chunks per core
DA = D + 1                # augmented row: centers row + csq

F32 = mybir.dt.float32
BF16 = mybir.dt.bfloat16
I32 = mybir.dt.int32

_CACHE = {}


def legalize_waits(nc, max_waits=1):
    """The walrus build in this container accepts at most one embedded
    sem-wait per TPB instruction ("Too many sync wait commands" otherwise).
    Split any excess into standalone single-wait InstEventSemaphore no-ops
    immediately before the instruction on the same engine — engine program
    order then enforces the identical synchronization."""
    n_split = 0
    for f in nc.m.functions:
        for b in f.blocks:
            insts = list(b.instructions)
            out = []
            for inst in insts:
                si = inst.sync_info
                waits = list(si.on_wait) if (si is not None and si.on_wait) else []
                if len(waits) > max_waits:
                    keep = waits[-max_waits:]
                    spill = waits[:-max_waits]
                    for k, w in enumerate(spill):
                        out.append(
                            mybir.InstEventSemaphore(
                                name=f"{inst.name}-lw{k}",
                                engine=inst.engine,
                                sync_info=mybir.SyncInfo(on_wait=[w], on_update=[]),
                            )
                        )
                        n_split += 1
                    inst.sync_info = mybir.SyncInfo(
                        on_wait=keep, on_update=list(si.on_update or [])
                    )
                out.append(inst)
            b.instructions = out
    return n_split


def embed_wait(bi, sem, val):
    """Attach a sem-ge wait directly to an instruction's sync_info instead of
    emitting a standalone EVENT_SEMAPHORE before it.  A standalone wait stalls
    the engine's prefetch of the next (large) instruction; an embedded wait
    lets fetch/decode/setup overlap the waiting."""
    ins = bi.ins
    si = ins.sync_info
    waits = list(si.on_wait or []) if si is not None else []
    upds = list(si.on_update or []) if si is not None else []
    waits.append(
        mybir.SyncWait(
            sync_type="semaphore",
            id=sem.num,
            ant_name=sem.name,
            wait_mode="sem-ge-imm",
            wait_value=val,
            wait_reg=None,
        )
    )
    ins.sync_info = mybir.SyncInfo(on_wait=waits, on_update=upds)
    return bi


def hoist_before_preamble(nc, inst_names):
    """Move the named instructions to the front of the main block, before the
    Bass-ctor const-AP memsets and all-engine barrier.  Only legal for
    instructions whose engine-side dependencies are register-free DMAs that
    touch no const APs: the owning engine then issues them ahead of its
    barrier arrival, overlapping the DMA latency with the preamble."""
    blk = nc.m.functions[0].blocks[0]
    insts = list(blk.instructions)
    moved = [i for i in insts if i.name in inst_names]
    rest = [i for i in insts if i.name not in inst_names]
    # keep the dummycall first (walrus uses it for the dge table)
    assert rest and type(rest[0]).__name__ == "InstCall"
    blk.instructions = [rest[0]] + moved + rest[1:]
    return len(moved)


def build_nc(centers_np):
    nc = bass.Bass(num_swdge_queues=2)

    x = nc.dram_tensor("x", [P, NT * DA], BF16, kind="ExternalInput")
    # labels pre-arranged on host: [p, t] = original label[t*128 + p]
    labels = nc.dram_tensor("labels", [P, NT], I32, kind="ExternalInput")
    out = nc.dram_tensor("out", [P, 1 + NT], F32, kind="ExternalOutput")

    cen = np.ascontiguousarray(centers_np, dtype=np.float32)
    csq = np.sum(cen * cen, axis=1, dtype=np.float32)
    cen_aug = np.concatenate([cen, csq[:, None]], axis=1).astype(ml_dtypes.bfloat16)
    centers = nc.inline_tensor(np.ascontiguousarray(cen_aug), name="centers")

    es = ExitStack()
    idx_sb = es.enter_context(nc.sbuf_tensor("idx_sb", [P, NT], I32))
    x_sb = es.enter_context(nc.sbuf_tensor("x_sb", [P, NT * DA], BF16))
    c_sb = es.enter_context(nc.sbuf_tensor("c_sb", [P, NT * DA], BF16))
    junka = es.enter_context(nc.sbuf_tensor("junka", [P, NT * DA], BF16))
    junkc = es.enter_context(nc.sbuf_tensor("junkc", [P, NT * DA], BF16))
    acc = es.enter_context(nc.sbuf_tensor("acc", [P, 1 + NT], F32))
    scr_sb = es.enter_context(nc.sbuf_tensor("scr_sb", [P, NT], I32))
    idx_sem = es.enter_context(nc.semaphore("idx_sem"))
    p_sem = es.enter_context(nc.semaphore("p_sem"))
    x_sem = es.enter_context(nc.semaphore("x_sem"))
    c_sems = [es.enter_context(nc.semaphore(f"c_sem{t}")) for t in range(NT)]
    a_sem = es.enter_context(nc.semaphore("a_sem"))
    v_sem = es.enter_context(nc.semaphore("v_sem"))
    o_sem = es.enter_context(nc.semaphore("o_sem"))
    dve_sem = es.enter_context(nc.semaphore("dve_sem"))
    f_sem = es.enter_context(nc.semaphore("f_sem"))

    # ---- HWDGE loads, hoisted pre-preamble.  ONE label DMA on the Sync
    # ring (splitting per chunk quadruples the tiny-descriptor packet storm
    # and delays the later columns' completion receipts by 1-3us); x alone
    # on the Scalar ring so the label receipts stay quiet. ----
    lab_dma = nc.sync.dma_start(out=idx_sb[:, :], in_=labels[:, :])
    lab_dma.then_inc(idx_sem, 16)
    x_dma = nc.scalar.dma_start(out=x_sb[:, :], in_=x[:, :])
    x_dma.then_inc(x_sem, 16)

    # ---- gpsimd: 4 indirect 128-row gathers, all on queue 0 so each
    # gather's completion receipts are flushed by the next one's
    # descriptors; one dummy chaser flushes the last. ----
    for t in range(NT):
        gi = nc.gpsimd.indirect_dma_start(
            out=c_sb[:, t * DA:(t + 1) * DA],
            out_offset=None,
            in_=centers[:],
            in_offset=bass.IndirectOffsetOnAxis(ap=idx_sb[:, t:t + 1], axis=0),
        ).then_inc(c_sems[t], 16)
        if t == 0:
            embed_wait(gi, idx_sem, 16)  # later gathers ride Pool order
    nc.gpsimd.dma_start(out=scr_sb[0:1, :], in_=labels[0:1, :]).then_inc(f_sem, 16)

    # ---- scalar/ACT (off critical path): acc0 = sum(x^2) + 1.0 ----
    embed_wait(nc.scalar.activation(
        out=junka[:, :],
        in_=x_sb[:, :],
        func=mybir.ActivationFunctionType.Square,
        scale=0.5,
        accum_out=acc[:, 0:1],
    ).then_inc(a_sem, 1), x_sem, 16)

    # ---- vector: one fused accumulate-STT per chunk (walrus rejects
    # TensorScalarPtr on the Pool engine, so no cross-engine split) ----
    for t in range(NT):
        stt = nc.vector.scalar_tensor_tensor(
            out=junkc[:, t * DA:(t + 1) * DA],
            in0=x_sb[:, t * DA:(t + 1) * DA],
            scalar=1.0,
            in1=c_sb[:, t * DA:(t + 1) * DA],
            op0=mybir.AluOpType.mult,
            op1=mybir.AluOpType.mult,
            accum_out=acc[:, 1 + t:2 + t],
        )
        if t == 0:
            # needs x AND the gather; the x wait spills standalone.
            embed_wait(stt, x_sem, 16)
        embed_wait(stt, c_sems[t], 16)
        stt.then_inc(v_sem if t == NT - 1 else dve_sem, 1)

    # ---- result out split across the two HWDGE rings.  v_sem >= 1 proves
    # the DVE STT accum drains (in-order DVE); p_sem the Pool half; a_sem
    # acc0.  Early waits spill standalone; the critical one stays embedded.
    H = P // 2
    for eng, rows in ((nc.sync, slice(0, H)), (nc.scalar, slice(H, P))):
        odma = eng.dma_start(out=out[rows, :], in_=acc[rows, :]).then_inc(o_sem, 16)
        embed_wait(odma, a_sem, 1)
        embed_wait(odma, v_sem, 1)

    # NOTE: the ExitStack is intentionally NOT closed — closing would free
    # the semaphores and emit an expensive end-of-program drain + barrier.
    hoist_before_preamble(nc, {lab_dma.ins.name, x_dma.ins.name})
    # Drop the ctor's all-engine barrier: the hoisted label/x DMAs delay
    # every engine's barrier arrival to ~8.8us, which in turn delays Pool's
    # first gather fetch past the label sems.  All cross-engine ordering in
    # this program is via data semaphores; the only barrier-protected state
    # is the const-AP memsets (used solely by ACT's bias const, first read
    # at x_sem ~11us >> memset completion ~6.7us).  The follower DRAINs
    # keep their embedded arrival updates (harmless; sems never reused).
    for f in nc.m.functions:
        for blk in f.blocks:
            blk.instructions = [
                i for i in blk.instructions if "barrier_" not in i.name
            ]
    legalize_waits(nc)
    return nc


def _get_nc(centers_np):
    arr = np.ascontiguousarray(centers_np, np.float32)
    key = hashlib.md5(arr.tobytes()).hexdigest()
    if _CACHE.get("key") != key:
        _CACHE["nc"] = build_nc(arr)
        _CACHE["key"] = key
    return _CACHE["nc"]


def make_in_maps(x, labels, centers=None):
    x = np.asarray(x, dtype=np.float32)
    # [p, t] = label[t*128 + p] within each core's 512-row shard
    labels_i32 = np.ascontiguousarray(
        np.asarray(labels).astype(np.int32).reshape(NCORES, NT, P).transpose(0, 2, 1)
    )
    # x transposed per core then augmented: [p, t*DA+d] = -2*x[t*128+p, d],
    # [p, t*DA+512] = 1.0
    xs = x.reshape(NCORES, NT, P, D).transpose(0, 2, 1, 3)  # [core, p, t, d]
    xa = np.empty((NCORES, P, NT, DA), dtype=np.float32)
    xa[..., :D] = -2.0 * xs
    xa[..., D] = 1.0
    xa = np.ascontiguousarray(
        xa.astype(ml_dtypes.bfloat16).reshape(NCORES, P, NT * DA)
    )
    return [{"x": xa[i], "labels": labels_i32[i]} for i in range(NCORES)]


def finalize(results):
    total = 0.0
    for r in results:
        vals = np.asarray(r["out"], dtype=np.float64)
        # acc0 = sum_t ||x_t||^2 + 1.0 (the four Square(0.5) constants),
        # acc[1..4] = csq_t - 2<x_t, c_t> per chunk.  The row clip never
        # binds (all distances ~1e3), so summing before the clip floor is
        # exact.
        total += float((vals[:, 0] - 1.0 + vals[:, 1:].sum(axis=1)).sum())
    loss = (total + B * (C - 1) * 1e-12) / B
    return np.array(loss, dtype=np.float32)


def kernel(x, labels, centers):
    nc = _get_nc(centers)
    in_maps = make_in_maps(x, labels)
    res = run_bass_kernel_spmd(nc, in_maps, core_ids=list(range(NCORES)))
    return finalize(res.results)
